# revision 1
# baseline (speedup 1.0000x reference)
"""Trainium2 Bass kernel for nn_CamFusionModule (epipolar max-sampling fusion).

Strategy
--------
Data-parallel over output pixels: the 64x64 heatmap grid is split into 8
row-bands of 8 rows, one per NeuronCore (heatmaps replicated, all 12
(curview, othview) pairs on every core, per the sharding hint's
"pair axis splittable / heatmaps replicated" guidance).

Host (jax-cpu, bit-identical to the reference):
  * camera math -> per-pair epipolar sweep coordinates, normalized,
    rounded and clamped exactly as the reference's grid_sample does ->
    fp16 index rows (one row per sweep position t).
  * heatmaps -> per-(pair, sweep, t-pair) stationary gather tables,
    split into fp16 (hi, lo) parts (hi+lo reconstructs ~21 bits), rows
    parity-interleaved (row k = table entry k//2 of sweep position
    2g + k%2) and block-diagonal over parity in the columns.

Device (per NeuronCore), per (pair, sweep):
  * index rows are replicated across all 128 partitions by a
    log-doubling chain of SBUF->SBUF DMAs (chunks of 8 t-pairs), so the
    idle DMA engines do the broadcast and the PE pstate ramp is not
    disturbed by tiny matmuls.
  * per t-pair, a one-hot mask [128, 512] = (P == k//2) is built either
    on DVE (`is_equal` vs a per-partition iota) or on ACT
    (Square(P - iota) -> Relu(1 - sq)), load-balanced.
  * two N=512 fp16 matmuls (hi, lo accumulating in PSUM) gather
    2 samples/column x 16 channels; outputs of 4 t-pairs are stacked
    into one full-width PSUM bank via 32-aligned col groups.
  * DVE running tensor-max over PSUM banks, then partition-block folds
    (small shift DMAs) collapse t-pair slots/parity/sweep.

Output: [12, 16, 512] fp32 per core, reassembled host-side.
"""

import numpy as np
import ml_dtypes

NVIEW = 4
B, C, H, W = 1, 16, 64, 64
HW = H * W
NPAIR = 12
NCORE = 8
PXS = HW // NCORE          # 512 pixels per core
ROWS = H // NCORE          # 8 image rows per core
NTP = W // 2               # 32 t-pairs per sweep
CHTP = 16                  # t-pairs per replication chunk
NCH = NTP // CHTP          # chunks per (pair, sweep)
BIG = 1.0e9                # sentinel for non-finite coords (-> invalid)
ACT_SHARE = 10             # of 32 t-pairs per (pair, sweep) masked on ScalarE

_PAIRS = [(c, o) for c in range(NVIEW) for o in range(NVIEW) if o != c]


def _line_coords(affine_trans, cam_Intri, cam_R, cam_T, inv_affine_trans):
    """Mirror of the reference's fp32 math through the rounded sample
    indices. Returns iy[p, t, px] (x-sweep row index) and ix[p, t, px]
    (y-sweep col index) as float32 [12, 64, 4096], exactly matching the
    reference's `jnp.round((g + 1) * 0.5 * (dim - 1))` values (jax on CPU
    so rounding matches bit-for-bit)."""
    import jax
    import jax.numpy as jnp
    cpu = jax.devices("cpu")[0]
    ctx = jax.default_device(cpu)
    ctx.__enter__()

    V = NVIEW
    h, w = H, W
    yy, xx = jnp.meshgrid(jnp.arange(h, dtype=jnp.float32),
                          jnp.arange(w, dtype=jnp.float32), indexing='ij')
    onehm = jnp.stack([xx.reshape(-1), yy.reshape(-1), jnp.ones(HW, jnp.float32)], 0)
    K = jnp.asarray(cam_Intri).reshape(B, V, 3, 3)
    R = jnp.asarray(cam_R).reshape(B, V, 3, 3)
    T = jnp.asarray(cam_T).reshape(B, V, 3, 1)
    Aff = jnp.asarray(affine_trans).reshape(B, V, 3, 3)
    invAff = jnp.asarray(inv_affine_trans).reshape(B, V, 3, 3)
    invK = jnp.linalg.inv(K)
    ray = jnp.einsum('bvij,bvjk,kp->bvip', invK, invAff, onehm)
    deps = jnp.array([1000.0, 5000.0], jnp.float32).reshape(2, 1, 1, 1, 1)
    xg = jnp.einsum('bvji,dbvjp->dbvip', R, deps * ray[None]) + T[None]
    xcam = jnp.einsum('boij,dbcojp->dbcoip', R, xg[:, :, :, None] - T[:, None])
    xnorm = xcam / xcam[:, :, :, :, 2:3]
    M = jnp.einsum('bvij,bvjk->bvik', Aff, K)
    uv = jnp.einsum('boij,dbcojp->dbcoip', M, xnorm)
    oth = np.array([[o for o in range(V) if o != c] for c in range(V)])
    uv = uv[:, :, jnp.arange(V)[:, None], oth]
    x0, y0 = uv[0, ..., 0, :], uv[0, ..., 1, :]
    x1, y1 = uv[1, ..., 0, :], uv[1, ..., 1, :]
    kk = (y1 - y0) / (x1 - x0)
    xs = jnp.arange(w, dtype=jnp.float32)
    ysw = kk[..., None] * (xs - x0[..., None]) + y0[..., None]   # (B,V,V-1,HW,w)
    ysh = jnp.arange(h, dtype=jnp.float32)
    xsh = (ysh - y0[..., None]) / kk[..., None] + x0[..., None]  # (B,V,V-1,HW,h)

    # Reference normalizes to [-1,1] then maps back before rounding; that
    # fp round-trip shifts values by a few ulp, so replicate it exactly.
    def _round_chain(v):
        v = jnp.where(jnp.isfinite(v), v, jnp.float32(BIG))
        g = v / jnp.float32((W - 1) / 2.0) - 1.0
        return jnp.round((g + 1.0) * 0.5 * (W - 1))

    iy = np.asarray(_round_chain(ysw), np.float32)
    ix = np.asarray(_round_chain(xsh), np.float32)
    iy = iy.reshape(NPAIR, HW, W).transpose(0, 2, 1)
    ix = ix.reshape(NPAIR, HW, H).transpose(0, 2, 1)
    ctx.__exit__(None, None, None)
    return iy, ix


def _host_indices(iy, ix):
    """clamp -> fp16 index rows [12, 2(sweep), 64(t), 4096(px)]."""
    out = np.empty((NPAIR, 2, W, HW), dtype=np.float16)
    for s, arr in enumerate((iy, ix)):
        r = np.clip(arr, -1.0, 64.0)           # invalid -> never matches iota
        r = np.where(np.isfinite(r), r, 64.0)  # NaN paranoia
        out[:, s] = r.astype(np.float16)
    return out


def _host_tables(heatmaps):
    """Parity-interleaved block-diagonal fp16 two-part gather tables.

    Returns [12, 2, 32, 128, 64] fp16. Row k holds table entry k//2 of
    sweep position t = 2g + (k % 2).  Columns:
      0:16  hi, even parity   16:32 hi, odd parity    (MM1 = cols 0:32)
      32:48 lo, even parity   48:64 lo, odd parity    (MM2 = cols 32:64)
    x-sweep entry (y, t) -> hm[o, ch, y, t]; y-sweep (x, t) -> hm[o, ch, t, x].
    """
    hm = np.asarray(heatmaps, np.float32).reshape(NVIEW, C, H, W)
    hi = hm.astype(np.float16)
    lo = (hm - hi.astype(np.float32)).astype(np.float16)

    tab = np.zeros((NPAIR, 2, NTP, 128, 64), dtype=np.float16)
    for p, (c, o) in enumerate(_PAIRS):
        for part, src in ((0, hi), (1, lo)):
            base = 32 * part
            xsv = src[o].transpose(2, 1, 0)   # [t, entry(y), ch]
            ysv = src[o].transpose(1, 2, 0)   # [t'(row), entry(x), ch]
            for sweep, v in ((0, xsv), (1, ysv)):
                # even parity: t = 2g, rows 0::2, cols base+0:16
                tab[p, sweep, :, 0::2, base + 0:base + 16] = v[0::2]
                # odd parity: t = 2g+1, rows 1::2, cols base+16:32
                tab[p, sweep, :, 1::2, base + 16:base + 32] = v[1::2]
    return tab


_COMPILED = {}


def _build_program():
    import concourse.bacc as bacc
    import concourse.mybir as mybir
    import concourse.tile as tile
    from contextlib import ExitStack

    dt = mybir.dt
    ops = mybir.AluOpType
    act = mybir.ActivationFunctionType

    nc = bacc.Bacc("TRN2", target_bir_lowering=False, debug=False,
                   num_devices=NCORE)

    # idxb: [pair, sweep, chunk, 32 replicated parity rows, g_local*512+px]
    idx_d = nc.dram_tensor("idxb", [NPAIR, 2, NCH, 32, CHTP * PXS], dt.float16,
                           kind="ExternalInput")
    tab_d = nc.dram_tensor("tab", [NPAIR, 2, NTP, 128, 64], dt.float16,
                           kind="ExternalInput")
    iota_d = nc.dram_tensor("iota", [128, 1], dt.float32, kind="ExternalInput")
    niota_d = nc.dram_tensor("niota", [128, 1], dt.float32, kind="ExternalInput")
    out_d = nc.dram_tensor("out", [NPAIR, 16, PXS], dt.float32,
                           kind="ExternalOutput")

    with tile.TileContext(nc) as tc:
        with ExitStack() as ctx:
            cpool = ctx.enter_context(tc.tile_pool(name="const", bufs=1))
            tpool = ctx.enter_context(tc.tile_pool(name="tabs", bufs=4))
            rpool = ctx.enter_context(tc.tile_pool(name="repl", bufs=6))
            mpool = ctx.enter_context(tc.tile_pool(name="mask", bufs=8))
            spool = ctx.enter_context(tc.tile_pool(name="sq", bufs=3))
            apool = ctx.enter_context(tc.tile_pool(name="acc", bufs=3))
            fpool = ctx.enter_context(tc.tile_pool(name="fold", bufs=3))
            espool = ctx.enter_context(tc.tile_pool(name="res", bufs=3))
            opool = ctx.enter_context(tc.tile_pool(name="O", bufs=5, space="PSUM"))

            iota = cpool.tile([128, 1], dt.float32, tag="iota")
            niota = cpool.tile([128, 1], dt.float32, tag="niota")
            nc.sync.dma_start(iota[:], iota_d.ap())
            nc.sync.dma_start(niota[:], niota_d.ap())

            for p in range(NPAIR):
                res_ps = None
                for s in range(2):
                    tab = tpool.tile([128, NTP * 64], dt.float16, tag="tab")
                    nc.sync.dma_start(
                        tab[:].rearrange("k (g x) -> k g x", g=NTP),
                        tab_d.ap()[p, s].rearrange("g k x -> k g x"))

                    # replicate idx rows chunk-wise via DMA doubling chains
                    reps = []
                    for cch in range(NCH):
                        rep = rpool.tile([128, CHTP * PXS], dt.float16,
                                         tag="rep")
                        nc.sync.dma_start(rep[0:32, :], idx_d.ap()[p, s, cch])
                        rr = 32
                        while rr < 128:
                            nc.sync.dma_start(rep[rr:2 * rr, :], rep[0:rr, :])
                            rr *= 2
                        reps.append(rep)

                    acc = apool.tile([128, PXS], dt.float32, tag="acc")
                    for gg in range(NTP // 4):
                        ops_ps = opool.tile([128, PXS], dt.float32, tag="O")
                        for slot in range(4):
                            g = gg * 4 + slot
                            rep = reps[g // CHTP]
                            gl = g % CHTP
                            P = rep[:, gl * PXS:(gl + 1) * PXS]
                            mask = mpool.tile([128, PXS], dt.float16, tag="m")
                            if g < ACT_SHARE:
                                sq = spool.tile([128, PXS], dt.float16,
                                                tag="sq")
                                nc.scalar.activation(sq[:], P, act.Square,
                                                     bias=niota[:], scale=1.0)
                                nc.scalar.activation(mask[:], sq[:], act.Relu,
                                                     bias=1.0, scale=-1.0)
                            else:
                                nc.vector.tensor_scalar(mask[:], P, iota[:],
                                                        None, ops.is_equal)
                            tsl = tab[:, g * 64:g * 64 + 32]
                            nc.tensor.matmul(
                                ops_ps[32 * slot:32 * slot + 32, :],
                                tsl, mask[:], start=True, stop=False,
                                tile_position=(0, 32 * slot))
                            tsl2 = tab[:, g * 64 + 32:g * 64 + 64]
                            nc.tensor.matmul(
                                ops_ps[32 * slot:32 * slot + 32, :],
                                tsl2, mask[:], start=False, stop=True,
                                tile_position=(0, 32 * slot))
                        if gg == 0:
                            nc.vector.tensor_copy(acc[:], ops_ps[:])
                        else:
                            nc.vector.tensor_tensor(acc[:], acc[:], ops_ps[:],
                                                    ops.max)
                    # fold 4 col-group slots (partition blocks of 32)
                    f64 = fpool.tile([64, PXS], dt.float32, tag="f64")
                    nc.scalar.dma_start(f64[:], acc[64:128, :])
                    nc.vector.tensor_tensor(f64[:], f64[:], acc[0:64, :], ops.max)
                    f32t = fpool.tile([32, PXS], dt.float32, tag="f32")
                    nc.scalar.dma_start(f32t[:], f64[32:64, :])
                    nc.vector.tensor_tensor(f32t[:], f32t[:], f64[0:32, :], ops.max)
                    # fold channel parity blocks (16)
                    f16 = fpool.tile([16, PXS], dt.float32, tag="f16")
                    nc.scalar.dma_start(f16[:], f32t[16:32, :])
                    nc.vector.tensor_tensor(f16[:], f16[:], f32t[0:16, :], ops.max)
                    if s == 0:
                        res_ps = espool.tile([16, PXS], dt.float32, tag="res")
                        nc.vector.tensor_copy(res_ps[:], f16[:])
                    else:
                        nc.vector.tensor_tensor(res_ps[:], res_ps[:], f16[:],
                                                ops.max)
                nc.sync.dma_start(out_d.ap()[p], res_ps[:])

    nc.compile()
    return nc


def _make_in_maps(inputs):
    iy, ix = _line_coords(inputs["affine_trans"], inputs["cam_Intri"],
                          inputs["cam_R"], inputs["cam_T"],
                          inputs["inv_affine_trans"])
    idx = _host_indices(iy, ix)             # [12, 2, 64, 4096] fp16
    tab = _host_tables(inputs["heatmaps"])  # [12, 2, 32, 128, 64] fp16

    iota = (np.arange(128, dtype=np.float32) // 2).reshape(128, 1)
    niota = np.ascontiguousarray(-iota)

    in_maps = []
    for i in range(NCORE):
        sl = slice(i * PXS, (i + 1) * PXS)
        idx_i = idx[:, :, :, sl]                       # [12, 2, 64t, 512]
        # [pair, sweep, chunk, parity, g_local, px] -> replicate parity rows x16
        idxb2 = np.ascontiguousarray(
            idx_i.reshape(NPAIR, 2, NCH, CHTP, 2, PXS).transpose(0, 1, 2, 4, 3, 5)
        ).reshape(NPAIR, 2, NCH, 1, 2, CHTP * PXS)
        idxb = np.ascontiguousarray(
            np.broadcast_to(idxb2, (NPAIR, 2, NCH, 16, 2, CHTP * PXS))
        ).reshape(NPAIR, 2, NCH, 32, CHTP * PXS)
        in_maps.append({"idxb": idxb, "tab": tab,
                        "iota": iota, "niota": niota})
    return in_maps


def kernel(heatmaps, affine_trans, cam_Intri, cam_R, cam_T, inv_affine_trans):
    from concourse.bass_utils import run_bass_kernel_spmd

    heatmaps = np.asarray(heatmaps)
    in_dtype = heatmaps.dtype
    inputs = {"heatmaps": heatmaps, "affine_trans": affine_trans,
              "cam_Intri": cam_Intri, "cam_R": cam_R, "cam_T": cam_T,
              "inv_affine_trans": inv_affine_trans}

    if "prog" not in _COMPILED:
        _COMPILED["prog"] = _build_program()
    nc = _COMPILED["prog"]

    in_maps = _make_in_maps(inputs)
    res = run_bass_kernel_spmd(nc, in_maps, list(range(NCORE)))

    out = np.empty((NVIEW, NVIEW - 1, C, H, W), dtype=np.float32)
    for i in range(NCORE):
        o_i = res.results[i]["out"].reshape(NPAIR, C, ROWS, W)
        for p, (c, o) in enumerate(_PAIRS):
            slot = [v for v in range(NVIEW) if v != c].index(o)
            out[c, slot, :, i * ROWS:(i + 1) * ROWS, :] = o_i[p]
    return out.reshape(NVIEW, NVIEW - 1, C, H, W).astype(in_dtype, copy=False)



# revision 4
# speedup vs baseline: 465.7906x; 465.7906x over previous
"""Trainium2 Bass kernel v2 for nn_CamFusionModule (epipolar max-sampling).

Architecture (per NeuronCore, 512 pixels = 8 heatmap rows):
  * Host precomputes, per (pair, sweep, t-pair), a one-hot fp8 gather mask
    [128, 512] (row k fires when the sweep index of t=2g+(k%2) equals k//2)
    and fp8 hi/lo split heatmap tables.  Masks stream from DRAM (2 MB per
    (pair, sweep) unit, one DMA each); zero on-device mask computation.
  * PE: one DoubleRow fp8 matmul per t-pair gathers hi and lo parts in a
    single pass (lhsT [128, 2, 32] = hi/lo weight sets; rhs = mask broadcast
    to [128, 2, 512]); 4 t-pairs col-tiled into one PSUM [128, 512] bank.
  * ScalarE copies each PSUM group to SBUF fp16; VectorE max-accumulates
    into a per-pair [128, 512] accumulator (some groups go straight to DVE
    from PSUM to balance engines), then 3 partition-fold steps (shift DMA +
    DVE max) collapse 4 slots x 2 parity -> [16, 512] fp32 output.
"""

import numpy as np
import ml_dtypes

NVIEW = 4
B, C, H, W = 1, 16, 64, 64
HW = H * W
NPAIR = 12
NCORE = 8
PXS = HW // NCORE          # 512 pixels per core
ROWS = H // NCORE          # 8 image rows per core
NTP = W // 2               # 32 t-pairs per sweep
NUNIT = NPAIR * 2          # 24 (pair, sweep) units
NGRP = NTP // 4            # 8 PSUM groups per unit (4 t-pairs each)
DVE_DIRECT = 2             # groups per unit max-accumulated straight from PSUM

F8 = ml_dtypes.float8_e4m3

_PAIRS = [(c, o) for c in range(NVIEW) for o in range(NVIEW) if o != c]


def _line_coords(affine_trans, cam_Intri, cam_R, cam_T, inv_affine_trans):
    """Bit-exact mirror of the reference's fp32 math through the rounded
    sample indices. Returns float32 [12, 2, 64, 4096]: [:,0] x-sweep row
    index iy, [:,1] y-sweep col index ix (non-finite -> huge -> invalid)."""
    import jax
    import jax.numpy as jnp
    cpu = jax.devices("cpu")[0]
    with jax.default_device(cpu):
        V = NVIEW
        h, w = H, W
        yy, xx = jnp.meshgrid(jnp.arange(h, dtype=jnp.float32),
                              jnp.arange(w, dtype=jnp.float32), indexing='ij')
        onehm = jnp.stack([xx.reshape(-1), yy.reshape(-1),
                           jnp.ones(HW, jnp.float32)], 0)
        K = jnp.asarray(cam_Intri).reshape(B, V, 3, 3)
        R = jnp.asarray(cam_R).reshape(B, V, 3, 3)
        T = jnp.asarray(cam_T).reshape(B, V, 3, 1)
        Aff = jnp.asarray(affine_trans).reshape(B, V, 3, 3)
        invAff = jnp.asarray(inv_affine_trans).reshape(B, V, 3, 3)
        invK = jnp.linalg.inv(K)
        ray = jnp.einsum('bvij,bvjk,kp->bvip', invK, invAff, onehm)
        deps = jnp.array([1000.0, 5000.0], jnp.float32).reshape(2, 1, 1, 1, 1)
        xg = jnp.einsum('bvji,dbvjp->dbvip', R, deps * ray[None]) + T[None]
        xcam = jnp.einsum('boij,dbcojp->dbcoip', R,
                          xg[:, :, :, None] - T[:, None])
        xnorm = xcam / xcam[:, :, :, :, 2:3]
        M = jnp.einsum('bvij,bvjk->bvik', Aff, K)
        uv = jnp.einsum('boij,dbcojp->dbcoip', M, xnorm)
        oth = np.array([[o for o in range(V) if o != c] for c in range(V)])
        uv = uv[:, :, jnp.arange(V)[:, None], oth]
        x0, y0 = uv[0, ..., 0, :], uv[0, ..., 1, :]
        x1, y1 = uv[1, ..., 0, :], uv[1, ..., 1, :]
        kk = (y1 - y0) / (x1 - x0)
        xs = jnp.arange(w, dtype=jnp.float32)
        ysw = kk[..., None] * (xs - x0[..., None]) + y0[..., None]
        ysh = jnp.arange(h, dtype=jnp.float32)
        xsh = (ysh - y0[..., None]) / kk[..., None] + x0[..., None]

        def _round_chain(v):
            v = jnp.where(jnp.isfinite(v), v, jnp.float32(1.0e9))
            g = v / jnp.float32((W - 1) / 2.0) - 1.0
            return jnp.round((g + 1.0) * 0.5 * (W - 1))

        iy = np.asarray(_round_chain(ysw), np.float32)
        ix = np.asarray(_round_chain(xsh), np.float32)
        iy = iy.reshape(NPAIR, HW, W).transpose(0, 2, 1)
        ix = ix.reshape(NPAIR, HW, H).transpose(0, 2, 1)
    return np.stack([iy, ix], 1)  # [12, 2, 64, 4096]


def _host_masks(idx):
    """One-hot fp8 masks per core: list of [128, 24, 32, 512] (viewed u8).

    idx: float32 [12, 2, 64, 4096]. Row k of mask (u=(p,s), g, px) is
    one(idx[p,s,2g+(k%2), px] == k//2)."""
    one = np.float32(1.0).astype(F8).view(np.uint8)  # fp8 bit pattern of 1.0
    finite = np.isfinite(idx)
    valid = finite & (idx >= 0) & (idx <= 63)
    iv = np.clip(np.nan_to_num(idx), 0, 63).astype(np.int64)
    # row index per sample (invalid -> dummy row 128)
    rows = np.where(valid, 2 * iv + (np.arange(64) % 2)[None, None, :, None], 128)
    rows = rows.reshape(NPAIR, 2, NTP, 2, HW)  # t = 2g+q -> (g, q)
    masks = []
    U, G_, PX = np.ogrid[0:NUNIT, 0:NTP, 0:PXS]
    for core in range(NCORE):
        sl = slice(core * PXS, (core + 1) * PXS)
        r = np.ascontiguousarray(
            rows[:, :, :, :, sl]).reshape(NUNIT, NTP, 2, PXS)
        m = np.zeros((129, NUNIT, NTP, PXS), np.uint8)
        m[r[:, :, 0], U, G_, PX] = one
        m[r[:, :, 1], U, G_, PX] = one
        masks.append(np.ascontiguousarray(m[:128]))
    return masks


def _host_tables(heatmaps):
    """fp8 hi/lo gather tables [128, 24, 32, 2, 32] (viewed u8).

    Row k = (entry e = k//2, parity q = k%2); for unit u=(p,s), t-pair g,
    weight set i (0 hi, 1 lo), col m = 16*q' + ch: value is part_i of
    hm[oth(p), ch, Y, X] with (Y, X) = (e, t) for s=0 and (t, e) for s=1,
    only when q == q' (parity block-diagonal), where t = 2g + q."""
    hm = np.asarray(heatmaps, np.float32).reshape(NVIEW, C, H, W)
    hi = hm.astype(F8)
    lo = (hm - hi.astype(np.float32)).astype(F8)
    hi8, lo8 = hi.view(np.uint8), lo.view(np.uint8)

    tab = np.zeros((128, NVIEW, 2, NTP, 2, 32), np.uint8)
    e = np.arange(64)
    g = np.arange(NTP)
    for o in range(NVIEW):
        for q in (0, 1):
            t = 2 * g + q                       # [32]
            csl = slice(16 * q, 16 * q + 16)
            # x-sweep (s=0): hm[o, ch, e, t] -> [64e, 32g, 16ch]
            tab[2 * e + q, o, 0, :, 0, csl] = hi8[o][:, :, t].transpose(1, 2, 0)
            tab[2 * e + q, o, 0, :, 1, csl] = lo8[o][:, :, t].transpose(1, 2, 0)
            # y-sweep (s=1): hm[o, ch, t, e] -> [64e, 32g, 16ch]
            tab[2 * e + q, o, 1, :, 0, csl] = hi8[o][:, t, :].transpose(2, 1, 0)
            tab[2 * e + q, o, 1, :, 1, csl] = lo8[o][:, t, :].transpose(2, 1, 0)
    return tab


_COMPILED = {}


def _build_program():
    import concourse.bacc as bacc
    import concourse.mybir as mybir
    import concourse.tile as tile
    from contextlib import ExitStack

    dt = mybir.dt
    ops = mybir.AluOpType
    DR = mybir.MatmulPerfMode.DoubleRow

    nc = bacc.Bacc("TRN2", target_bir_lowering=False, debug=False,
                   num_devices=NCORE)

    masks_d = nc.dram_tensor("masks", [128, NUNIT, NTP, PXS], dt.float8e4,
                             kind="ExternalInput")
    tabs_d = nc.dram_tensor("tabs", [128, NVIEW, 2, NTP, 2, 32], dt.float8e4,
                            kind="ExternalInput")
    out_d = nc.dram_tensor("out", [NPAIR, 16, PXS], dt.float32,
                           kind="ExternalOutput")

    with tile.TileContext(nc) as tc:
        with ExitStack() as ctx:
            tbpool = ctx.enter_context(tc.tile_pool(name="tabs", bufs=1))
            mpool = ctx.enter_context(tc.tile_pool(name="masks", bufs=3))
            gpool = ctx.enter_context(tc.tile_pool(name="grp", bufs=6))
            apool = ctx.enter_context(tc.tile_pool(name="acc", bufs=2))
            fpool = ctx.enter_context(tc.tile_pool(name="fold", bufs=2))
            opool = ctx.enter_context(tc.tile_pool(name="out", bufs=1))
            ppool = ctx.enter_context(tc.tile_pool(name="ps", bufs=6,
                                                   space="PSUM"))

            tabs = tbpool.tile([128, NVIEW * 2 * NTP * 2 * 32],
                               dt.float8e4, tag="tabs")
            nc.sync.dma_start(
                tabs[:].rearrange("k (o s g i m) -> k o s g i m",
                                  o=NVIEW, s=2, g=NTP, i=2),
                tabs_d.ap())

            # Zero-padded weight windows: per (o,s) unit-kind, per t-pair g
            # a private 256-byte window [i(2), m(128)] whose only nonzero
            # block sits at cols [32*(g%4), 32*(g%4)+32) -- so 4 accumulating
            # M=128 DoubleRow matmuls stack 4 t-pairs into one PSUM bank
            # without tile_position (the ISA check rejects DoubleRow+tiling).
            # Pool (idle engine) zeroes and scatters on demand; pairs are
            # processed grouped by their oth view so only 2 stay resident.
            pdpool = ctx.enter_context(tc.tile_pool(name="padt", bufs=2))
            pad_cache = {}

            def padded_tab(os_):
                if os_ in pad_cache:
                    return pad_cache[os_]
                padt = pdpool.tile([128, NTP * 256 // 4], dt.float32,
                                   tag="padt")
                p8 = padt[:].bitcast(dt.float8e4)   # [128, 32*256] fp8
                nc.gpsimd.memset(padt[:], 0)
                src = tabs[:, os_ * 2048:(os_ + 1) * 2048].rearrange(
                    "k (go s4 i m) -> k go s4 i m", go=8, s4=4, i=2)
                dst = p8.rearrange("k (go x) -> k go x", go=8)
                for s4 in range(4):
                    for i in range(2):
                        c = s4 * 288 + i * 128
                        nc.gpsimd.tensor_copy(dst[:, :, c:c + 32],
                                              src[:, :, s4, i])
                if len(pad_cache) >= 2:
                    del pad_cache[next(iter(pad_cache))]
                pad_cache[os_] = p8
                return p8

            out_t = opool.tile([16, NPAIR * PXS], dt.float32, tag="out")
            pair_order = sorted(range(NPAIR), key=lambda p: _PAIRS[p][1])
            for p in pair_order:
                acc = apool.tile([128, PXS], dt.float16, tag="acc")
                first = True
                for s in range(2):
                    u = 2 * p + s
                    mt = mpool.tile([128, NTP * PXS], dt.float8e4, tag="m")
                    nc.sync.dma_start(
                        mt[:].rearrange("k (g x) -> k g x", g=NTP),
                        masks_d.ap()[:, u])
                    os_ = _PAIRS[p][1] * 2 + s
                    pad8 = padded_tab(os_)
                    for grp in range(NGRP):
                        ps = ppool.tile([128, PXS], dt.float32, tag="ps")
                        for slot in range(4):
                            g = grp * 4 + slot
                            woff = g * 256
                            lhsT = pad8[:, woff:woff + 256].rearrange(
                                "k (i m) -> k i m", i=2)
                            rhs = mt[:, g * PXS:(g + 1) * PXS] \
                                .unsqueeze(1).broadcast_to([128, 2, PXS])
                            nc.tensor.matmul(
                                ps[:, :], lhsT, rhs,
                                start=(slot == 0), stop=(slot == 3),
                                perf_mode=DR)
                        if first:
                            nc.scalar.copy(acc[:], ps[:])
                            first = False
                        elif grp >= NGRP - DVE_DIRECT:
                            nc.vector.tensor_tensor(acc[:], acc[:], ps[:],
                                                    ops.max)
                        else:
                            gt = gpool.tile([128, PXS], dt.float16, tag="g")
                            nc.scalar.copy(gt[:], ps[:])
                            nc.vector.tensor_tensor(acc[:], acc[:], gt[:],
                                                    ops.max)
                # partition folds: 128 (8 slots x 16 ch) -> 64 -> 32 -> 16
                f64 = fpool.tile([64, PXS], dt.float16, tag="f64")
                nc.sync.dma_start(f64[:], acc[64:128, :])
                nc.vector.tensor_tensor(f64[:], f64[:], acc[0:64, :], ops.max)
                f32t = fpool.tile([32, PXS], dt.float16, tag="f32")
                nc.sync.dma_start(f32t[:], f64[32:64, :])
                nc.vector.tensor_tensor(f32t[:], f32t[:], f64[0:32, :],
                                        ops.max)
                f16 = fpool.tile([16, PXS], dt.float16, tag="f16")
                nc.sync.dma_start(f16[:], f32t[16:32, :])
                nc.vector.tensor_tensor(out_t[:, p * PXS:(p + 1) * PXS],
                                        f16[:], f32t[0:16, :], ops.max)

            nc.sync.dma_start(
                out_d.ap().rearrange("p c x -> c p x"),
                out_t[:].rearrange("c (p x) -> c p x", p=NPAIR))

    nc.compile()
    return nc


def _make_in_maps(inputs):
    idx = _line_coords(inputs["affine_trans"], inputs["cam_Intri"],
                       inputs["cam_R"], inputs["cam_T"],
                       inputs["inv_affine_trans"])
    masks = _host_masks(idx)
    tabs = _host_tables(inputs["heatmaps"])
    return [{"masks": masks[i].view(F8), "tabs": tabs.view(F8)}
            for i in range(NCORE)]


def _assemble(results, in_dtype=np.float32):
    out = np.empty((NVIEW, NVIEW - 1, C, H, W), dtype=np.float32)
    for i in range(NCORE):
        o_i = results[i]["out"].reshape(NPAIR, C, ROWS, W)
        for p, (c, o) in enumerate(_PAIRS):
            slot = [v for v in range(NVIEW) if v != c].index(o)
            out[c, slot, :, i * ROWS:(i + 1) * ROWS, :] = o_i[p]
    return out.reshape(NVIEW, NVIEW - 1, C, H, W).astype(in_dtype, copy=False)


def kernel(heatmaps, affine_trans, cam_Intri, cam_R, cam_T, inv_affine_trans):
    from concourse.bass_utils import run_bass_kernel_spmd

    heatmaps = np.asarray(heatmaps)
    in_dtype = heatmaps.dtype
    inputs = {"heatmaps": heatmaps, "affine_trans": affine_trans,
              "cam_Intri": cam_Intri, "cam_R": cam_R, "cam_T": cam_T,
              "inv_affine_trans": inv_affine_trans}

    if "prog" not in _COMPILED:
        _COMPILED["prog"] = _build_program()
    nc = _COMPILED["prog"]

    in_maps = _make_in_maps(inputs)
    res = run_bass_kernel_spmd(nc, in_maps, list(range(NCORE)))
    return _assemble(res.results, in_dtype)


# revision 6
# speedup vs baseline: 468.7817x; 1.0064x over previous
"""Trainium2 Bass kernel v2 for nn_CamFusionModule (epipolar max-sampling).

Architecture (per NeuronCore, 512 pixels = 8 heatmap rows):
  * Host precomputes, per (pair, sweep, t-pair), a one-hot fp8 gather mask
    [128, 512] (row k fires when the sweep index of t=2g+(k%2) equals k//2)
    and fp8 hi/lo split heatmap tables.  Masks stream from DRAM (2 MB per
    (pair, sweep) unit, one DMA each); zero on-device mask computation.
  * PE: one DoubleRow fp8 matmul per t-pair gathers hi and lo parts in a
    single pass (lhsT [128, 2, 32] = hi/lo weight sets; rhs = mask broadcast
    to [128, 2, 512]); 4 t-pairs col-tiled into one PSUM [128, 512] bank.
  * ScalarE copies each PSUM group to SBUF fp16; VectorE max-accumulates
    into a per-pair [128, 512] accumulator (some groups go straight to DVE
    from PSUM to balance engines), then 3 partition-fold steps (shift DMA +
    DVE max) collapse 4 slots x 2 parity -> [16, 512] fp32 output.
"""

import numpy as np
import ml_dtypes

NVIEW = 4
B, C, H, W = 1, 16, 64, 64
HW = H * W
NPAIR = 12
NCORE = 8
PXS = HW // NCORE          # 512 pixels per core
ROWS = H // NCORE          # 8 image rows per core
NTP = W // 2               # 32 t-pairs per sweep
NUNIT = NPAIR * 2          # 24 (pair, sweep) units
NGRP = NTP // 4            # 8 PSUM groups per unit (4 t-pairs each)
DVE_DIRECT = 2             # groups per unit max-accumulated straight from PSUM

F8 = ml_dtypes.float8_e4m3

_PAIRS = [(c, o) for c in range(NVIEW) for o in range(NVIEW) if o != c]


def _line_coords(affine_trans, cam_Intri, cam_R, cam_T, inv_affine_trans):
    """Bit-exact mirror of the reference's fp32 math through the rounded
    sample indices. Returns float32 [12, 2, 64, 4096]: [:,0] x-sweep row
    index iy, [:,1] y-sweep col index ix (non-finite -> huge -> invalid)."""
    import jax
    import jax.numpy as jnp
    cpu = jax.devices("cpu")[0]
    with jax.default_device(cpu):
        V = NVIEW
        h, w = H, W
        yy, xx = jnp.meshgrid(jnp.arange(h, dtype=jnp.float32),
                              jnp.arange(w, dtype=jnp.float32), indexing='ij')
        onehm = jnp.stack([xx.reshape(-1), yy.reshape(-1),
                           jnp.ones(HW, jnp.float32)], 0)
        K = jnp.asarray(cam_Intri).reshape(B, V, 3, 3)
        R = jnp.asarray(cam_R).reshape(B, V, 3, 3)
        T = jnp.asarray(cam_T).reshape(B, V, 3, 1)
        Aff = jnp.asarray(affine_trans).reshape(B, V, 3, 3)
        invAff = jnp.asarray(inv_affine_trans).reshape(B, V, 3, 3)
        invK = jnp.linalg.inv(K)
        ray = jnp.einsum('bvij,bvjk,kp->bvip', invK, invAff, onehm)
        deps = jnp.array([1000.0, 5000.0], jnp.float32).reshape(2, 1, 1, 1, 1)
        xg = jnp.einsum('bvji,dbvjp->dbvip', R, deps * ray[None]) + T[None]
        xcam = jnp.einsum('boij,dbcojp->dbcoip', R,
                          xg[:, :, :, None] - T[:, None])
        xnorm = xcam / xcam[:, :, :, :, 2:3]
        M = jnp.einsum('bvij,bvjk->bvik', Aff, K)
        uv = jnp.einsum('boij,dbcojp->dbcoip', M, xnorm)
        oth = np.array([[o for o in range(V) if o != c] for c in range(V)])
        uv = uv[:, :, jnp.arange(V)[:, None], oth]
        x0, y0 = uv[0, ..., 0, :], uv[0, ..., 1, :]
        x1, y1 = uv[1, ..., 0, :], uv[1, ..., 1, :]
        kk = (y1 - y0) / (x1 - x0)
        xs = jnp.arange(w, dtype=jnp.float32)
        ysw = kk[..., None] * (xs - x0[..., None]) + y0[..., None]
        ysh = jnp.arange(h, dtype=jnp.float32)
        xsh = (ysh - y0[..., None]) / kk[..., None] + x0[..., None]

        def _round_chain(v):
            v = jnp.where(jnp.isfinite(v), v, jnp.float32(1.0e9))
            g = v / jnp.float32((W - 1) / 2.0) - 1.0
            return jnp.round((g + 1.0) * 0.5 * (W - 1))

        iy = np.asarray(_round_chain(ysw), np.float32)
        ix = np.asarray(_round_chain(xsh), np.float32)
        iy = iy.reshape(NPAIR, HW, W).transpose(0, 2, 1)
        ix = ix.reshape(NPAIR, HW, H).transpose(0, 2, 1)
    return np.stack([iy, ix], 1)  # [12, 2, 64, 4096]


def _host_masks(idx):
    """One-hot fp8 masks per core: list of [128, 24, 32, 512] (viewed u8).

    idx: float32 [12, 2, 64, 4096]. Row k of mask (u=(p,s), g, px) is
    one(idx[p,s,2g+(k%2), px] == k//2)."""
    one = np.float32(1.0).astype(F8).view(np.uint8)  # fp8 bit pattern of 1.0
    finite = np.isfinite(idx)
    valid = finite & (idx >= 0) & (idx <= 63)
    iv = np.clip(np.nan_to_num(idx), 0, 63).astype(np.int64)
    # row index per sample (invalid -> dummy row 128)
    rows = np.where(valid, 2 * iv + (np.arange(64) % 2)[None, None, :, None], 128)
    rows = rows.reshape(NPAIR, 2, NTP, 2, HW)  # t = 2g+q -> (g, q)
    masks = []
    U, G_, PX = np.ogrid[0:NUNIT, 0:NTP, 0:PXS]
    for core in range(NCORE):
        sl = slice(core * PXS, (core + 1) * PXS)
        r = np.ascontiguousarray(
            rows[:, :, :, :, sl]).reshape(NUNIT, NTP, 2, PXS)
        m = np.zeros((129, NUNIT, NTP, PXS), np.uint8)
        m[r[:, :, 0], U, G_, PX] = one
        m[r[:, :, 1], U, G_, PX] = one
        masks.append(np.ascontiguousarray(m[:128]))
    return masks


def _host_tables(heatmaps):
    """fp8 hi/lo gather tables [128, 24, 32, 2, 32] (viewed u8).

    Row k = (entry e = k//2, parity q = k%2); for unit u=(p,s), t-pair g,
    weight set i (0 hi, 1 lo), col m = 16*q' + ch: value is part_i of
    hm[oth(p), ch, Y, X] with (Y, X) = (e, t) for s=0 and (t, e) for s=1,
    only when q == q' (parity block-diagonal), where t = 2g + q."""
    hm = np.asarray(heatmaps, np.float32).reshape(NVIEW, C, H, W)
    hi = hm.astype(F8)
    lo = (hm - hi.astype(np.float32)).astype(F8)
    hi8, lo8 = hi.view(np.uint8), lo.view(np.uint8)

    tab = np.zeros((128, NVIEW, 2, NTP, 2, 32), np.uint8)
    e = np.arange(64)
    g = np.arange(NTP)
    for o in range(NVIEW):
        for q in (0, 1):
            t = 2 * g + q                       # [32]
            csl = slice(16 * q, 16 * q + 16)
            # x-sweep (s=0): hm[o, ch, e, t] -> [64e, 32g, 16ch]
            tab[2 * e + q, o, 0, :, 0, csl] = hi8[o][:, :, t].transpose(1, 2, 0)
            tab[2 * e + q, o, 0, :, 1, csl] = lo8[o][:, :, t].transpose(1, 2, 0)
            # y-sweep (s=1): hm[o, ch, t, e] -> [64e, 32g, 16ch]
            tab[2 * e + q, o, 1, :, 0, csl] = hi8[o][:, t, :].transpose(2, 1, 0)
            tab[2 * e + q, o, 1, :, 1, csl] = lo8[o][:, t, :].transpose(2, 1, 0)
    return tab


_COMPILED = {}


def _build_program():
    import concourse.bacc as bacc
    import concourse.mybir as mybir
    import concourse.tile as tile
    from contextlib import ExitStack

    dt = mybir.dt
    ops = mybir.AluOpType
    DR = mybir.MatmulPerfMode.DoubleRow

    nc = bacc.Bacc("TRN2", target_bir_lowering=False, debug=False,
                   num_devices=NCORE)

    masks_d = nc.dram_tensor("masks", [128, NUNIT, NTP, PXS], dt.float8e4,
                             kind="ExternalInput")
    tabs_d = nc.dram_tensor("tabs", [128, NVIEW, 2, NTP, 2, 32], dt.float8e4,
                            kind="ExternalInput")
    out_d = nc.dram_tensor("out", [NPAIR, 16, PXS], dt.float32,
                           kind="ExternalOutput")

    with tile.TileContext(nc) as tc:
        with ExitStack() as ctx:
            tbpool = ctx.enter_context(tc.tile_pool(name="tabs", bufs=1))
            mpool = ctx.enter_context(tc.tile_pool(name="masks", bufs=3))
            gpool = ctx.enter_context(tc.tile_pool(name="grp", bufs=6))
            apool = ctx.enter_context(tc.tile_pool(name="acc", bufs=2))
            fpool = ctx.enter_context(tc.tile_pool(name="fold", bufs=2))
            opool = ctx.enter_context(tc.tile_pool(name="out", bufs=1))
            ppool = ctx.enter_context(tc.tile_pool(name="ps", bufs=6,
                                                   space="PSUM"))

            tabs = tbpool.tile([128, NVIEW * 2 * NTP * 2 * 32],
                               dt.float8e4, tag="tabs")
            nc.sync.dma_start(
                tabs[:].rearrange("k (o s g i m) -> k o s g i m",
                                  o=NVIEW, s=2, g=NTP, i=2),
                tabs_d.ap())

            # Zero-padded weight windows: per (o,s) unit-kind, per t-pair g
            # a private 256-byte window [i(2), m(128)] whose only nonzero
            # block sits at cols [32*(g%4), 32*(g%4)+32) -- so 4 accumulating
            # M=128 DoubleRow matmuls stack 4 t-pairs into one PSUM bank
            # without tile_position (the ISA check rejects DoubleRow+tiling).
            # Pool (idle engine) zeroes and scatters on demand; pairs are
            # processed grouped by their oth view so only 2 stay resident.
            pdpool = ctx.enter_context(tc.tile_pool(name="padt", bufs=2))
            pad_cache = {}

            def padded_tab(os_):
                if os_ in pad_cache:
                    return pad_cache[os_]
                padt = pdpool.tile([128, NTP * 256 // 4], dt.float32,
                                   tag="padt")
                p8 = padt[:].bitcast(dt.float8e4)   # [128, 32*256] fp8
                nc.gpsimd.memset(padt[:], 0)
                src = tabs[:, os_ * 2048:(os_ + 1) * 2048].rearrange(
                    "k (go s4 i m) -> k go s4 i m", go=8, s4=4, i=2)
                dst = p8.rearrange("k (go x) -> k go x", go=8)
                for s4 in range(4):
                    for i in range(2):
                        c = s4 * 288 + i * 128
                        nc.gpsimd.tensor_copy(dst[:, :, c:c + 32],
                                              src[:, :, s4, i])
                if len(pad_cache) >= 2:
                    del pad_cache[next(iter(pad_cache))]
                pad_cache[os_] = p8
                return p8

            out_t = opool.tile([16, NPAIR * PXS], dt.float32, tag="out")
            pair_order = sorted(range(NPAIR), key=lambda p: _PAIRS[p][1])
            for p in pair_order:
                acc = apool.tile([128, PXS], dt.float16, tag="acc")
                first = True
                for s in range(2):
                    u = 2 * p + s
                    mt = mpool.tile([128, NTP * PXS], dt.float8e4, tag="m")
                    nc.sync.dma_start(
                        mt[:].rearrange("k (g x) -> k g x", g=NTP),
                        masks_d.ap()[:, u])
                    os_ = _PAIRS[p][1] * 2 + s
                    pad8 = padded_tab(os_)
                    for grp in range(NGRP):
                        ps = ppool.tile([128, PXS], dt.float32, tag="ps")
                        for slot in range(4):
                            g = grp * 4 + slot
                            woff = g * 256
                            lhsT = pad8[:, woff:woff + 256].rearrange(
                                "k (i m) -> k i m", i=2)
                            rhs = mt[:, g * PXS:(g + 1) * PXS] \
                                .unsqueeze(1).broadcast_to([128, 2, PXS])
                            nc.tensor.matmul(
                                ps[:, :], lhsT, rhs,
                                start=(slot == 0), stop=(slot == 3),
                                perf_mode=DR)
                        if first:
                            nc.scalar.copy(acc[:], ps[:])
                            first = False
                        elif grp >= NGRP - DVE_DIRECT:
                            nc.vector.tensor_tensor(acc[:], acc[:], ps[:],
                                                    ops.max)
                        else:
                            gt = gpool.tile([128, PXS], dt.float16, tag="g")
                            nc.scalar.copy(gt[:], ps[:])
                            nc.vector.tensor_tensor(acc[:], acc[:], gt[:],
                                                    ops.max)
                # partition folds: 128 (8 slots x 16 ch) -> 64 -> 32 -> 16
                f64 = fpool.tile([64, PXS], dt.float16, tag="f64")
                nc.sync.dma_start(f64[:], acc[64:128, :])
                nc.vector.tensor_tensor(f64[:], f64[:], acc[0:64, :], ops.max)
                f32t = fpool.tile([32, PXS], dt.float16, tag="f32")
                nc.sync.dma_start(f32t[:], f64[32:64, :])
                nc.vector.tensor_tensor(f32t[:], f32t[:], f64[0:32, :],
                                        ops.max)
                f16 = fpool.tile([16, PXS], dt.float16, tag="f16")
                nc.sync.dma_start(f16[:], f32t[16:32, :])
                nc.vector.tensor_tensor(out_t[:, p * PXS:(p + 1) * PXS],
                                        f16[:], f32t[0:16, :], ops.max)

            last = pair_order[-1]
            ot3 = out_t[:].rearrange("c (p x) -> c p x", p=NPAIR)
            od3 = out_d.ap().rearrange("p c x -> c p x")
            if last > 0:
                nc.sync.dma_start(od3[:, 0:last], ot3[:, 0:last])
            if last < NPAIR - 1:
                nc.sync.dma_start(od3[:, last + 1:], ot3[:, last + 1:])
            nc.sync.dma_start(od3[:, last:last + 1], ot3[:, last:last + 1])

    nc.compile()
    return nc


def _make_in_maps(inputs):
    idx = _line_coords(inputs["affine_trans"], inputs["cam_Intri"],
                       inputs["cam_R"], inputs["cam_T"],
                       inputs["inv_affine_trans"])
    masks = _host_masks(idx)
    tabs = _host_tables(inputs["heatmaps"])
    return [{"masks": masks[i].view(F8), "tabs": tabs.view(F8)}
            for i in range(NCORE)]


def _assemble(results, in_dtype=np.float32):
    out = np.empty((NVIEW, NVIEW - 1, C, H, W), dtype=np.float32)
    for i in range(NCORE):
        o_i = results[i]["out"].reshape(NPAIR, C, ROWS, W)
        for p, (c, o) in enumerate(_PAIRS):
            slot = [v for v in range(NVIEW) if v != c].index(o)
            out[c, slot, :, i * ROWS:(i + 1) * ROWS, :] = o_i[p]
    return out.reshape(NVIEW, NVIEW - 1, C, H, W).astype(in_dtype, copy=False)


def kernel(heatmaps, affine_trans, cam_Intri, cam_R, cam_T, inv_affine_trans):
    from concourse.bass_utils import run_bass_kernel_spmd

    heatmaps = np.asarray(heatmaps)
    in_dtype = heatmaps.dtype
    inputs = {"heatmaps": heatmaps, "affine_trans": affine_trans,
              "cam_Intri": cam_Intri, "cam_R": cam_R, "cam_T": cam_T,
              "inv_affine_trans": inv_affine_trans}

    if "prog" not in _COMPILED:
        _COMPILED["prog"] = _build_program()
    nc = _COMPILED["prog"]

    in_maps = _make_in_maps(inputs)
    res = run_bass_kernel_spmd(nc, in_maps, list(range(NCORE)))
    return _assemble(res.results, in_dtype)
